# revision 1
# baseline (speedup 1.0000x reference)
"""GCN layer (2 edge types, mean aggregation + self-loop) on 8 Trainium2 cores.

Math (per reference):
    m_t = segment_mean(h[src_t] @ Wt.T, dst_t)   for t in {1,2}
    out = relu(h @ Wl.T + bl + 0.5*(m1 + m2))

Key identity: linear commutes with gather+mean, so we aggregate raw h rows
(segment-mean) first and apply the 128x128 weights afterwards:
    m_t = segment_mean(h[src_t], dst_t) @ Wt.T

Sharding: destination nodes are partitioned contiguously across 8 cores.
Edges are routed host-side to the core owning their dst. Each core's dst
range is processed in 128-row "blocks" (one block = one schedule "slot");
edges of one block are consumed in chunks of 128 via an indicator matmul
accumulated in PSUM:
    s_block[d, f] += sum_e ind[e, d] * g[e, f]
where ind[e, d] = (dst_rel[e] == d), built on-chip with a tensor_scalar
is_equal against an iota row, and g = gathered h rows for the chunk\'s edges.

The gather uses the native GPSIMD dma_gather (int16 indices), so h is split
into 4 banks of <=32768 rows; the chunk schedule is bank-major:
    for bank: for slot: for type: cap[t][slot][bank] chunks
Within one bank the gather calls cover long runs of consecutive chunks
(KG chunks per call).  Unfilled index slots gather bank row 0 (they cost
bandwidth but keep every call\'s index count static, which the shared SPMD
instruction stream requires); their dst_rel sentinel (255) zeroes them in
the indicator, so they contribute nothing.  Per-(slot,type) partial sums
accumulate in PSUM within one bank pass and are added into an SBUF
accumulator across bank passes.

All 8 cores share one instruction stream (SPMD): the capacity profile
cap[t][s][b] is the max over cores, each core permutes its blocks onto
slots (sorted by edge count) to keep the profile tight, and the output is
un-permuted on the host.

h is gathered from a packed bf16 hi/lo table ([N, 256]: cols 0:128 = bf16(h),
128:256 = bf16(h - hi)), giving 512B gather rows (full DMA line rate) and
~f32 precision via two accumulating matmuls per chunk.  The final weight
matmuls run as float32r on slot *pairs* (256-wide outputs) for full PE rate.
"""

import numpy as np
import ml_dtypes

BF16 = np.dtype(ml_dtypes.bfloat16)

# ---------------------------------------------------------------- config ---

N_NODES = 100000
HIDDEN = 128
N_CORES = 8
ROWS_PER_CORE = N_NODES // N_CORES  # 12500
BANK = 32768     # dma_gather int16 index range
KG = 4           # chunks per dma_gather call (<=1024 descriptors: SWDGE ring limit)
PAD_DREL = 255.0  # dst_rel sentinel for padded edge slots -> indicator 0


def _cdiv(a, b):
    return -(-a // b)


# ------------------------------------------------------------ host routing ---

def _route(srcs, dsts, rows_per_core, n_cores, n_nodes):
    """Compute per-core tables + shared (bank, slot, type) chunk schedule."""
    n_types = len(srcs)
    S_real = _cdiv(rows_per_core, 128)
    S = S_real + (S_real % 2)  # pad to even for slot-pairing
    NB = _cdiv(n_nodes, BANK)

    counts = np.zeros((n_cores, n_types, S, NB), np.int64)
    core_of, block_of, drel_of, bank_of = [], [], [], []
    for t in range(n_types):
        dst = dsts[t].astype(np.int64)
        src = srcs[t].astype(np.int64)
        c = dst // rows_per_core
        dl = dst - c * rows_per_core
        b = dl // 128
        bk = src // BANK
        core_of.append(c)
        block_of.append(b)
        bank_of.append(bk)
        drel_of.append((dl - b * 128).astype(np.float32))
        np.add.at(counts, (c, t, b, bk), 1)

    # per-core block->slot permutation (sorted by type-0 count desc)
    key = counts[:, 0, :, :].sum(axis=2)
    perms = np.argsort(-key, axis=1, kind="stable")
    inv_perms = np.argsort(perms, axis=1)

    sorted_counts = np.take_along_axis(counts, perms[:, None, :, None], axis=2)
    caps = _cdiv(sorted_counts, 128).max(axis=0)  # [n_types, S, NB]
    # ensure every (t, s) has >= 1 chunk so its sacc region gets written
    empty_ts = caps.sum(axis=2) == 0
    if empty_ts.any():
        ti, si = np.nonzero(empty_ts)
        caps[ti, si, 0] = 1

    # chunk layout (bank-major)
    chunk_base = np.zeros((n_types, S, NB), np.int64)
    pos = 0
    bank_cols = []
    for b in range(NB):
        c0 = pos
        for s in range(S):
            for t in range(n_types):
                chunk_base[t, s, b] = pos
                pos += int(caps[t, s, b])
        bank_cols.append((c0, pos))
    n_chunks = pos

    # gather calls: per bank, runs of KG chunks
    calls = []  # (bank, col0, width)
    for b, (c0, c1) in enumerate(bank_cols):
        c = c0
        while c < c1:
            w = min(KG, c1 - c)
            calls.append((b, c, w))
            c += w

    invdeg = []
    for t in range(n_types):
        deg = np.bincount(dsts[t].astype(np.int64),
                          minlength=rows_per_core * n_cores)
        invdeg.append((1.0 / np.maximum(deg, 1)).astype(np.float32))

    per_core = []
    for c in range(n_cores):
        flat_idx = np.zeros(n_chunks * 128, np.int16)  # pad = bank row 0
        drel = np.full((128, n_chunks), PAD_DREL, np.float32)
        inv = np.ones((n_types, 128, S), np.float32)
        for t in range(n_types):
            mask = core_of[t] == c
            e_idx = np.nonzero(mask)[0]
            slots = inv_perms[c][block_of[t][e_idx]]
            banks = bank_of[t][e_idx]
            # group by (bank, slot); sort by src within for HBM locality
            order = np.lexsort((srcs[t][e_idx], slots, banks))
            e_idx = e_idx[order]
            slots = slots[order]
            banks = banks[order]
            gkey = banks * S + slots
            uniq, start = np.unique(gkey, return_index=True)
            start = np.append(start, len(e_idx))
            for gi, g in enumerate(uniq):
                bk, s = int(g) // S, int(g) % S
                lo, hi = start[gi], start[gi + 1]
                base = chunk_base[t, s, bk] * 128
                posn = base + np.arange(hi - lo)
                flat_idx[posn] = (srcs[t][e_idx[lo:hi]] - bk * BANK
                                  ).astype(np.int16)
                drel[posn % 128, posn // 128] = drel_of[t][e_idx[lo:hi]]
            # inverse degree table in slot order
            blk = perms[c]
            node = c * rows_per_core + blk[None, :] * 128 + \
                np.arange(128)[:, None]
            valid = (blk[None, :] * 128 + np.arange(128)[:, None]) \
                < rows_per_core
            ok = valid & (blk[None, :] < S_real)
            node = np.where(ok, node, 0)
            inv[t] = np.where(ok, invdeg[t][node], 1.0)

        # wrapped int16 index table: flat i -> partition i%16 (replicated
        # across the 8 groups of 16 partitions), column i//16
        gidx_cols = []
        for (bk, col0, w) in calls:
            seg = flat_idx[col0 * 128:(col0 + w) * 128]
            wrapped = seg.reshape(-1, 16).T  # [16, w*8]
            gidx_cols.append(np.tile(wrapped, (8, 1)))
        gidx = np.ascontiguousarray(np.concatenate(gidx_cols, axis=1))
        per_core.append(dict(gidx=gidx, drel=drel, inv=inv, perm=perms[c]))

    return dict(caps=caps, n_chunks=n_chunks, S=S, S_real=S_real, NB=NB,
                calls=calls, chunk_base=chunk_base, per_core=per_core)


# ------------------------------------------------------------ bass program ---

def _build_program(rt, n_nodes, n_cores, reps=1):
    """Build the SPMD bass program (shared by all cores)."""
    import concourse.bacc as bacc
    from concourse import mybir, tile, library_config

    caps, S, NB = rt["caps"], rt["S"], rt["NB"]
    n_chunks, calls, chunk_base = rt["n_chunks"], rt["calls"], rt["chunk_base"]
    n_types = caps.shape[0]
    F = HIDDEN
    nc = bacc.Bacc("TRN2", target_bir_lowering=False, debug=False,
                   num_devices=n_cores)
    dt = mybir.dt

    hpk = nc.dram_tensor("hpk", [n_nodes, 2 * F], dt.bfloat16,
                         kind="ExternalInput").ap()
    gidx_d = nc.dram_tensor("gidx", [128, n_chunks * 8], dt.int16,
                            kind="ExternalInput").ap()
    drel_d = nc.dram_tensor("drel", [128, n_chunks], dt.float32,
                            kind="ExternalInput").ap()
    inv_d = [nc.dram_tensor(f"inv{t}", [128, S], dt.float32,
                            kind="ExternalInput").ap() for t in range(n_types)]
    hot_d = nc.dram_tensor("hot", [128, S * 128], dt.float32r,
                           kind="ExternalInput").ap()
    w_d = [nc.dram_tensor(w, [128, 128], dt.float32r,
                          kind="ExternalInput").ap()
           for w in ("w1t", "w2t", "wlt")]
    blc_d = nc.dram_tensor("blc", [128, 1], dt.float32,
                           kind="ExternalInput").ap()
    iota_d = nc.dram_tensor("iota", [128, 128], dt.bfloat16,
                            kind="ExternalInput").ap()
    outT_d = nc.dram_tensor("outT", [128, S * 128], dt.float32,
                            kind="ExternalOutput").ap()

    # first/last bank with nonzero cap per (t, s)
    first_bank, last_bank = {}, {}
    for t in range(n_types):
        for s in range(S):
            nz = [b for b in range(NB) if caps[t, s, b] > 0]
            first_bank[(t, s)] = nz[0]
            last_bank[(t, s)] = nz[-1]

    chunk_info = [None] * n_chunks
    for b in range(NB):
        for s in range(S):
            for t in range(n_types):
                for q in range(int(caps[t, s, b])):
                    ci = int(chunk_base[t, s, b]) + q
                    chunk_info[ci] = (b, s, t, q, int(caps[t, s, b]))
    call_of_chunk = {}
    for k, (bk, col0, w) in enumerate(calls):
        for ci in range(col0, col0 + w):
            call_of_chunk[ci] = (k, col0, w)

    with tile.TileContext(nc) as tc:
        with (
            tc.tile_pool(name="const", bufs=1) as const_p,
            tc.tile_pool(name="gpool", bufs=12) as gpool,
            tc.tile_pool(name="ind", bufs=3) as ind_p,
            tc.tile_pool(name="mslot", bufs=2) as m_p,
            tc.tile_pool(name="mpair", bufs=2) as mt_p,
            tc.tile_pool(name="hot", bufs=2) as hot_p,
            tc.tile_pool(name="ostage", bufs=2) as o_p,
            tc.tile_pool(name="ps0", bufs=2, space="PSUM") as ps0_p,
            tc.tile_pool(name="ps1", bufs=2, space="PSUM") as ps1_p,
            tc.tile_pool(name="psT", bufs=2, space="PSUM") as psT_p,
            tc.tile_pool(name="pso", bufs=2, space="PSUM") as pso_p,
        ):
            nc.gpsimd.load_library(library_config.mlp)
            gidx_s = const_p.tile([128, n_chunks * 8], dt.int16, name="gidx_s")
            nc.sync.dma_start(out=gidx_s[:], in_=gidx_d[:, :])
            drel_s = const_p.tile([128, n_chunks], dt.float32, name="drel_s")
            nc.sync.dma_start(out=drel_s[:], in_=drel_d[:, :])
            inv_s = []
            for t in range(n_types):
                it = const_p.tile([128, S], dt.float32, tag=f"inv{t}",
                                  name=f"invs{t}")
                nc.sync.dma_start(out=it[:], in_=inv_d[t][:, :])
                inv_s.append(it)
            w_s = []
            for i, wd in enumerate(w_d):
                wt = const_p.tile([128, 128], dt.float32r, tag=f"w{i}",
                                  name=f"ws{i}")
                nc.sync.dma_start(out=wt[:], in_=wd[:, :])
                w_s.append(wt)
            blc_s = const_p.tile([128, 1], dt.float32, name="blc_s")
            nc.sync.dma_start(out=blc_s[:], in_=blc_d[:, :])
            iota_s = const_p.tile([128, 128], dt.bfloat16, name="iota_s")
            nc.sync.dma_start(out=iota_s[:], in_=iota_d[:, :])
            eye_s = const_p.tile([128, 128], dt.float32, name="eye_s")
            from concourse.masks import make_identity
            make_identity(nc, eye_s[:])

            sacc = [const_p.tile([128, S * 128], dt.float32, tag=f"sacc{t}",
                                 name=f"sacc{t}") for t in range(n_types)]

            f32r = dt.float32r
            relu = mybir.ActivationFunctionType.Relu
            iseq = mybir.AluOpType.is_equal
            mult = mybir.AluOpType.mult

            for rep in range(reps):
                cur_ps = {}
                cur_mT = [None]

                def finalize_slot(s):
                    if s % 2 == 0:
                        cur_mT[0] = [
                            mt_p.tile([128, 256], f32r, tag=f"mt{t}",
                                      name=f"mt{t}") for t in range(n_types)]
                    half = (s % 2) * 128
                    for t in range(n_types):
                        m = m_p.tile([128, 128], dt.float32, tag=f"m{t}",
                                     name=f"m{t}")
                        nc.vector.tensor_scalar(
                            out=m[:], in0=sacc[t][:, s * 128:(s + 1) * 128],
                            scalar1=inv_s[t][:, s:s + 1], scalar2=None,
                            op0=mult)
                        pt = psT_p.tile([128, 128], dt.float32, tag="pt",
                                        name="pt")
                        nc.tensor.transpose(out=pt[:], in_=m[:],
                                            identity=eye_s[:])
                        nc.vector.tensor_copy(
                            out=cur_mT[0][t][:, half:half + 128], in_=pt[:])
                    if s % 2 == 1:
                        q2 = s // 2
                        hot_t = hot_p.tile([128, 256], f32r, tag="hot",
                                           name="hot_t")
                        nc.sync.dma_start(
                            out=hot_t[:],
                            in_=hot_d[:, q2 * 256:(q2 + 1) * 256])
                        pso = pso_p.tile([128, 256], dt.float32, tag="pso",
                                         name="pso")
                        nc.tensor.matmul(out=pso[:], lhsT=w_s[0][:],
                                         rhs=cur_mT[0][0][:],
                                         start=True, stop=False)
                        nc.tensor.matmul(out=pso[:], lhsT=w_s[1][:],
                                         rhs=cur_mT[0][1][:],
                                         start=False, stop=False)
                        nc.tensor.matmul(out=pso[:], lhsT=w_s[2][:],
                                         rhs=hot_t[:],
                                         start=False, stop=True)
                        ot = o_p.tile([128, 256], dt.float32, tag="ot",
                                      name="ot")
                        nc.scalar.activation(out=ot[:], in_=pso[:], func=relu,
                                             bias=blc_s[:, 0:1])
                        nc.sync.dma_start(
                            out=outT_d[:, q2 * 256:(q2 + 1) * 256], in_=ot[:])

                g_tile = None
                for ci in range(n_chunks):
                    b, s, t, q, cap = chunk_info[ci]
                    k, col0, w = call_of_chunk[ci]
                    if ci == col0:
                        bk0 = calls[k][0] * BANK
                        bk1 = min(bk0 + BANK, n_nodes)
                        g_tile = gpool.tile([128, KG, 2 * F], dt.bfloat16,
                                            tag="g", name="g")
                        nc.gpsimd.dma_gather(
                            g_tile[:, :w, :], hpk[bk0:bk1, :],
                            gidx_s[:, col0 * 8:(col0 + w) * 8],
                            128 * w, 128 * w, 2 * F,
                            single_packet=False)
                    jj = ci - col0
                    ind = ind_p.tile([128, 128], dt.bfloat16, tag="ind",
                                     name="ind")
                    nc.vector.tensor_scalar(
                        out=ind[:], in0=iota_s[:],
                        scalar1=drel_s[:, ci:ci + 1], scalar2=None, op0=iseq)
                    if q == 0:
                        cur_ps[t] = (ps0_p if t == 0 else ps1_p).tile(
                            [128, 128], dt.float32, tag=f"ps{t}",
                            name=f"ps{t}")
                    ps = cur_ps[t]
                    nc.tensor.matmul(out=ps[:], lhsT=ind[:],
                                     rhs=g_tile[:, jj, 0:F],
                                     start=(q == 0), stop=False)
                    nc.tensor.matmul(out=ps[:], lhsT=ind[:],
                                     rhs=g_tile[:, jj, F:2 * F],
                                     start=False, stop=(q == cap - 1))
                    if q == cap - 1:
                        cols = slice(s * 128, (s + 1) * 128)
                        if b == first_bank[(t, s)]:
                            nc.vector.tensor_copy(out=sacc[t][:, cols],
                                                  in_=ps[:])
                        else:
                            nc.vector.tensor_add(out=sacc[t][:, cols],
                                                 in0=sacc[t][:, cols],
                                                 in1=ps[:])

                for s in range(S):
                    finalize_slot(s)

    nc.compile()
    return nc


# ------------------------------------------------------------------ driver ---

def _prepare(h, src1, dst1, src2, dst2, W1, W2, Wl, bl,
             rows_per_core, n_cores):
    """Host-side packing. Returns (route, in_maps)."""
    h = np.asarray(h, np.float32)
    bl = np.asarray(bl, np.float32)
    srcs = [np.asarray(src1), np.asarray(src2)]
    dsts = [np.asarray(dst1), np.asarray(dst2)]
    n_nodes = h.shape[0]
    rt = _route(srcs, dsts, rows_per_core, n_cores, n_nodes)
    S, S_real = rt["S"], rt["S_real"]

    hi = h.astype(BF16)
    lo = (h - hi.astype(np.float32)).astype(BF16)
    hpk = np.concatenate([hi, lo], axis=1)  # [N, 256] bf16

    w1t = (0.5 * np.asarray(W1, np.float32).T).copy()
    w2t = (0.5 * np.asarray(W2, np.float32).T).copy()
    wlt = np.asarray(Wl, np.float32).T.copy()
    blc = bl.reshape(128, 1).copy()
    iota = np.broadcast_to(np.arange(128, dtype=np.float32), (128, 128))
    iota = np.ascontiguousarray(iota.astype(BF16))

    in_maps = []
    for c in range(n_cores):
        pc = rt["per_core"][c]
        rows = h[c * rows_per_core:(c + 1) * rows_per_core]
        pad = S * 128 - rows.shape[0]
        rows = np.pad(rows, ((0, pad), (0, 0)))
        blocks = rows.reshape(S, 128, HIDDEN)[pc["perm"]]
        hot = np.ascontiguousarray(
            blocks.transpose(2, 0, 1).reshape(HIDDEN, S * 128))
        in_maps.append(dict(
            hpk=hpk, gidx=pc["gidx"], drel=pc["drel"],
            inv0=np.ascontiguousarray(pc["inv"][0]),
            inv1=np.ascontiguousarray(pc["inv"][1]),
            hot=hot, w1t=w1t, w2t=w2t, wlt=wlt, blc=blc, iota=iota,
        ))
    return rt, in_maps


def _postprocess(results, rt, rows_per_core, n_cores):
    n_nodes = rows_per_core * n_cores
    out = np.empty((n_nodes, HIDDEN), np.float32)
    for c in range(n_cores):
        outT = results[c]["outT"]  # [128, S*128]
        perm = rt["per_core"][c]["perm"]
        for s, b in enumerate(perm):
            lo_r = b * 128
            if lo_r >= rows_per_core:
                continue
            width = min(128, rows_per_core - lo_r)
            out[c * rows_per_core + lo_r:
                c * rows_per_core + lo_r + width] = \
                outT[:, s * 128:s * 128 + width].T
    return out


def kernel(h, src1, dst1, src2, dst2, W1, W2, Wl, bl, **kw):
    from concourse import bass_utils
    rt, in_maps = _prepare(h, src1, dst1, src2, dst2, W1, W2, Wl, bl,
                           ROWS_PER_CORE, N_CORES)
    nc = _build_program(rt, N_NODES, N_CORES)
    res = bass_utils.run_bass_kernel_spmd(
        nc, in_maps, core_ids=list(range(N_CORES)))
    return _postprocess(res.results, rt, ROWS_PER_CORE, N_CORES)



# revision 7
# speedup vs baseline: 2.3690x; 2.3690x over previous
"""GCN layer (2 edge types, mean aggregation + self-loop) on 8 Trainium2 cores.

Math (per reference):
    m_t = segment_mean(h[src_t] @ Wt.T, dst_t)   for t in {1,2}
    out = relu(h @ Wl.T + bl + 0.5*(m1 + m2))

Linear commutes with gather+mean, so raw h rows are segment-mean'd first
and the 128x128 weights applied afterwards.

Design (v2) — the kernel is dma_gather-descriptor-bound, so everything
is organized around minimizing gather descriptors and parallelizing the
SWDGE queues:

  * dst nodes partitioned contiguously across 8 cores; each core's 12500
    rows = 98 slots of 128.  Slots are processed in 9 PSUM-resident
    groups (8x12 + 2): each (type, slot) keeps a [128f x 128d] f32 PSUM
    accumulator alive across all 4 src windows, so there is no SBUF
    accumulator traffic at all.
  * The segment-sum is a *flipped* indicator matmul
        psT[f, d] += sum_e g[e, f] * ind[e, d],
    i.e. matmul(lhsT=g_chunk, rhs=ind) — this directly yields the
    transposed mean that the final weight matmul wants as rhs, removing
    the per-slot transposes of v1.  ind is built on DVE in one fused
    tensor_scalar: (iota == drel) * invdeg  (both per-partition scalars),
    so the mean's 1/deg scale rides along for free.
  * Edges are routed to (core, type, group, window) cells and packed
    *densely* into chunks of 128 (cross-slot chunks; a chunk spanning
    several slots emits one indicator+matmul pair per spanned slot).
    src indices are int16 relative to one of 4 windows of 25000 rows.
  * h is gathered as packed bf16 hi/lo rows ([N, 256] bf16: cols 0:128 =
    bf16(h), 128:256 = bf16(h - hi)) -> 512B/descriptor, ~f32 precision
    via two accumulating matmuls per (chunk, slot).
  * Gather calls are KG=4 chunks (512 descriptors) spread round-robin
    over 4 SWDGE queues (measured ~3-10x faster than one queue).
  * Final per slot-pair: psT copied PSUM->SBUF on the Activation engine,
    then 3 float32r matmuls (256-wide for full PE rate):
        out.T = relu(0.5*W1.T' m1T + 0.5*W2.T' m2T + Wl.T' hT + bl)

All 8 cores share one SPMD instruction stream; the chunk/subchunk
schedule is the max-shape over cores, per-core tables (gather indices,
drel/inv scalar columns) specialize it.  Padding slots gather window row
0 and carry a sentinel drel -> indicator 0.
"""

import numpy as np
import ml_dtypes

BF16 = np.dtype(ml_dtypes.bfloat16)

# ---------------------------------------------------------------- config ---

N_NODES = 100000
HIDDEN = 128
N_CORES = 8
ROWS_PER_CORE = N_NODES // N_CORES  # 12500
S = 98                        # dst slots per core (12544 >= 12500)
GROUP_SIZES = [12] * 8 + [2]  # PSUM-resident slot groups
NW = 4                        # src windows
WBASE = 25000                 # window w covers rows [w*WBASE, (w+1)*WBASE)
KG = 4                        # chunks per dma_gather call
NQ = 4                        # SWDGE queues
SCRATCH = 32768               # dynamic DMA descriptor carveout
SENT = 999.0                  # drel sentinel -> indicator 0


def _cdiv(a, b):
    return -(-a // b)


# ------------------------------------------------------------ host routing ---

def _route(srcs, dsts):
    """Build the shared (static) chunk/subchunk schedule + per-core tables."""
    NG = len(GROUP_SIZES)
    grp_base = np.concatenate([[0], np.cumsum(GROUP_SIZES)[:-1]])
    grp_of = np.repeat(np.arange(NG), GROUP_SIZES)  # slot -> group

    n_types = len(srcs)
    # per-type inverse degree over all nodes
    invdeg = []
    for t in range(n_types):
        deg = np.bincount(dsts[t].astype(np.int64), minlength=N_NODES)
        invdeg.append((1.0 / np.maximum(deg, 1)).astype(np.float32))

    # per-edge fields + per-core sorted orders
    ed = []
    for t in range(n_types):
        src = srcs[t].astype(np.int64)
        dst = dsts[t].astype(np.int64)
        c = dst // ROWS_PER_CORE
        dl = dst - c * ROWS_PER_CORE
        s = dl >> 7
        d128 = (dl & 127).astype(np.float32)
        g = grp_of[s]
        w = src // WBASE
        idx16 = (src - w * WBASE).astype(np.int16)
        order = np.lexsort((src, s, w, g, c))
        ed.append(dict(c=c[order], s=s[order], d128=d128[order],
                       g=g[order], w=w[order], idx16=idx16[order],
                       inv=invdeg[t][dst[order]]))

    # cell counts: cnt[t][c, g, w] and per-slot cnt_s[t][c, g, w, s_loc]
    gmax = max(GROUP_SIZES)
    cnt = np.zeros((n_types, N_CORES, NG, NW), np.int64)
    cnt_s = np.zeros((n_types, N_CORES, NG, NW, gmax), np.int64)
    for t in range(n_types):
        e = ed[t]
        s_loc = e["s"] - grp_base[e["g"]]
        np.add.at(cnt[t], (e["c"], e["g"], e["w"]), 1)
        np.add.at(cnt_s[t], (e["c"], e["g"], e["w"], s_loc), 1)

    # chunk layout: cells ordered (g, w, t); caps = max over cores
    caps = np.zeros((NG, NW, n_types), np.int64)
    for g in range(NG):
        for w in range(NW):
            for t in range(n_types):
                caps[g, w, t] = _cdiv(int(cnt[t][:, g, w].max()), 128)
    # coverage guard: every (t, global slot) needs >=1 possible subchunk;
    # ensure each (g, t) has at least one chunk somewhere
    for g in range(NG):
        for t in range(n_types):
            if caps[g, :, t].sum() == 0:
                caps[g, 0, t] = 1

    chunk_base = np.zeros((NG, NW, n_types), np.int64)
    pos = 0
    for g in range(NG):
        for w in range(NW):
            for t in range(n_types):
                chunk_base[g, w, t] = pos
                pos += int(caps[g, w, t])
    n_chunks = pos

    # gather calls: per (g, w) contiguous chunk run, split into <=KG pieces
    calls = []  # (window, col0, width)
    for g in range(NG):
        for w in range(NW):
            c0 = int(chunk_base[g, w, 0])
            c1 = int(chunk_base[g, w, n_types - 1] + caps[g, w, n_types - 1])
            c = c0
            while c < c1:
                wd = min(KG, c1 - c)
                calls.append((w, c, wd))
                c += wd

    # subchunk schedule: per chunk, union over cores of spanned local slots
    subs_of_chunk = [[] for _ in range(n_chunks)]
    for g in range(NG):
        gsz = GROUP_SIZES[g]
        for w in range(NW):
            for t in range(n_types):
                Q = int(caps[g, w, t])
                if Q == 0:
                    continue
                base = int(chunk_base[g, w, t])
                sets = [set() for _ in range(Q)]
                for c in range(N_CORES):
                    cum = 0
                    for sl in range(gsz):
                        n = int(cnt_s[t][c, g, w, sl])
                        if n == 0:
                            continue
                        q0, q1 = cum // 128, (cum + n - 1) // 128
                        for q in range(q0, q1 + 1):
                            sets[q].add(sl)
                        cum += n
                for q in range(Q):
                    subs_of_chunk[base + q] = sorted(sets[q])

    # coverage injection for any (t, s) with no subchunk
    covered = np.zeros((n_types, S), bool)
    for g in range(NG):
        for w in range(NW):
            for t in range(n_types):
                base = int(chunk_base[g, w, t])
                for q in range(int(caps[g, w, t])):
                    for sl in subs_of_chunk[base + q]:
                        covered[t, grp_base[g] + sl] = True
    for t in range(n_types):
        for s in range(S):
            if not covered[t, s]:
                g = int(grp_of[s])
                for w in range(NW):
                    if caps[g, w, t] > 0:
                        base = int(chunk_base[g, w, t])
                        sl = s - int(grp_base[g])
                        subs_of_chunk[base].append(sl)
                        subs_of_chunk[base] = sorted(subs_of_chunk[base])
                        break

    # global subchunk ids + start/stop flags per (t, global slot)
    sub_id = {}     # (chunk, s_loc) -> j
    chunk_cell = [None] * n_chunks  # chunk -> (g, w, t)
    j = 0
    first_j = {}
    last_j = {}
    for g in range(NG):
        for w in range(NW):
            for t in range(n_types):
                base = int(chunk_base[g, w, t])
                for q in range(int(caps[g, w, t])):
                    ci = base + q
                    chunk_cell[ci] = (g, w, t)
                    for sl in subs_of_chunk[ci]:
                        sub_id[(ci, sl)] = j
                        key = (t, int(grp_base[g]) + sl)
                        if key not in first_j:
                            first_j[key] = j
                        last_j[key] = j
                        j += 1
    n_sub = j

    # ------------------------------------------------------ per-core tables
    per_core = []
    for c in range(N_CORES):
        flat_idx = np.zeros(n_chunks * 128, np.int16)
        scl = np.empty((128, 2 * n_sub), np.float32)
        scl[:, 0::2] = SENT
        scl[:, 1::2] = 1.0
        for t in range(n_types):
            e = ed[t]
            mask = e["c"] == c
            idx = np.nonzero(mask)[0]
            if len(idx) == 0:
                continue
            gs, ws = e["g"][idx], e["w"][idx]
            ss = e["s"][idx]
            sl = ss - grp_base[gs]
            # edges are already sorted (g, w, s, src) within this core
            cellkey = gs * NW + ws
            bounds = np.nonzero(np.diff(cellkey))[0] + 1
            starts = np.concatenate([[0], bounds])
            ends = np.concatenate([bounds, [len(idx)]])
            for lo, hi in zip(starts, ends):
                g, w = int(gs[lo]), int(ws[lo])
                base = int(chunk_base[g, w, t])
                p = np.arange(hi - lo)
                eidx = idx[lo:hi]
                flat_idx[base * 128 + p] = e["idx16"][eidx]
                # per slot-run, fill scl columns
                slr = sl[lo:hi]
                rb = np.nonzero(np.diff(slr))[0] + 1
                rst = np.concatenate([[0], rb])
                ren = np.concatenate([rb, [hi - lo]])
                for a, b in zip(rst, ren):
                    s_loc = int(slr[a])
                    q0, q1 = a // 128, (b - 1) // 128
                    for q in range(q0, q1 + 1):
                        pa, pb = max(a, q * 128), min(b, (q + 1) * 128)
                        jj = sub_id[(base + q, s_loc)]
                        rows = np.arange(pa, pb) % 128
                        sel = eidx[pa:pb]
                        scl[rows, 2 * jj] = e["d128"][sel]
                        scl[rows, 2 * jj + 1] = e["inv"][sel]

        # wrapped int16 gather-index table, per call
        gidx = np.zeros((128, n_chunks * 8), np.int16)
        for (w, col0, wd) in calls:
            seg = flat_idx[col0 * 128:(col0 + wd) * 128]
            wrapped = seg.reshape(-1, 16).T
            gidx[:, col0 * 8:(col0 + wd) * 8] = np.tile(wrapped, (8, 1))
        per_core.append(dict(gidx=np.ascontiguousarray(gidx), scl=scl))

    return dict(caps=caps, chunk_base=chunk_base, n_chunks=n_chunks,
                n_sub=n_sub, calls=calls, subs_of_chunk=subs_of_chunk,
                sub_id=sub_id, chunk_cell=chunk_cell,
                first_j=first_j, last_j=last_j,
                grp_base=grp_base, per_core=per_core)


# ------------------------------------------------------------ bass program ---

def _build_program(rt, n_nodes, n_cores, reps=1):
    import concourse.bacc as bacc
    from concourse import mybir, tile, library_config

    NG = len(GROUP_SIZES)
    n_types = 2
    caps, chunk_base = rt["caps"], rt["chunk_base"]
    n_chunks, n_sub = rt["n_chunks"], rt["n_sub"]
    calls, subs_of_chunk = rt["calls"], rt["subs_of_chunk"]
    sub_id, first_j, last_j = rt["sub_id"], rt["first_j"], rt["last_j"]
    grp_base = rt["grp_base"]

    nc = bacc.Bacc("TRN2", target_bir_lowering=False, debug=False,
                   num_devices=n_cores, dynamic_dma_scratch_size=SCRATCH,
                   num_swdge_queues=NQ)
    dt = mybir.dt

    hpk = nc.dram_tensor("hpk", [n_nodes, 256], dt.bfloat16,
                         kind="ExternalInput").ap()
    gidx_d = nc.dram_tensor("gidx", [128, n_chunks * 8], dt.int16,
                            kind="ExternalInput").ap()
    scl_d = nc.dram_tensor("scl", [128, 2 * n_sub], dt.float32,
                           kind="ExternalInput").ap()
    hot_d = nc.dram_tensor("hot", [128, S * 128], dt.float32r,
                           kind="ExternalInput").ap()
    w_d = [nc.dram_tensor(w, [128, 128], dt.float32r,
                          kind="ExternalInput").ap()
           for w in ("w1t", "w2t", "wlt")]
    blc_d = nc.dram_tensor("blc", [128, 1], dt.float32,
                           kind="ExternalInput").ap()
    iota_d = nc.dram_tensor("iota", [128, 128], dt.bfloat16,
                            kind="ExternalInput").ap()
    outT_d = nc.dram_tensor("outT", [128, S * 128], dt.float32,
                            kind="ExternalOutput").ap()

    # per-chunk static info: (cell, call window base)
    call_of_chunk = {}
    for k, (w, col0, wd) in enumerate(calls):
        for ci in range(col0, col0 + wd):
            call_of_chunk[ci] = k

    # PSUM bank (= quad of 4 slots) first/last subchunk ids.  A matmul
    # start clears accumulate-bits for the WHOLE bank, so only the bank's
    # globally-first matmul may set start (per-element bits then make the
    # first write to every other column range an overwrite), and only the
    # bank's globally-last matmul sets stop.
    first_q, last_q = {}, {}
    for (t, s), j0 in first_j.items():
        g = int(np.searchsorted(np.append(grp_base, S), s, side="right") - 1)
        qkey = (t, g, (s - int(grp_base[g])) // 4)
        first_q[qkey] = min(first_q.get(qkey, j0), j0)
    for (t, s), j1 in last_j.items():
        g = int(np.searchsorted(np.append(grp_base, S), s, side="right") - 1)
        qkey = (t, g, (s - int(grp_base[g])) // 4)
        last_q[qkey] = max(last_q.get(qkey, j1), j1)

    with tile.TileContext(nc) as tc:
        with (
            tc.tile_pool(name="const", bufs=1) as const_p,
            tc.tile_pool(name="gpool", bufs=4) as gpool,
            tc.tile_pool(name="ind", bufs=6) as ind_p,
            tc.tile_pool(name="mt", bufs=2) as mt_p,
            tc.tile_pool(name="hot", bufs=2) as hot_p,
            tc.tile_pool(name="ostage", bufs=2) as o_p,
            tc.tile_pool(name="psT", bufs=1, space="PSUM") as psT_p,
            tc.tile_pool(name="pso", bufs=2, space="PSUM") as pso_p,
        ):
            nc.gpsimd.load_library(library_config.mlp)
            gidx_s = const_p.tile([128, n_chunks * 8], dt.int16, name="gidx_s")
            nc.sync.dma_start(out=gidx_s[:], in_=gidx_d[:, :])
            scl_s = const_p.tile([128, 2 * n_sub], dt.float32, name="scl_s")
            nc.sync.dma_start(out=scl_s[:], in_=scl_d[:, :])
            w_s = []
            for i, wd_ in enumerate(w_d):
                wt = const_p.tile([128, 128], dt.float32r, tag=f"w{i}",
                                  name=f"ws{i}")
                nc.sync.dma_start(out=wt[:], in_=wd_[:, :])
                w_s.append(wt)
            blc_s = const_p.tile([128, 1], dt.float32, name="blc_s")
            nc.sync.dma_start(out=blc_s[:], in_=blc_d[:, :])
            iota_s = const_p.tile([128, 128], dt.bfloat16, name="iota_s")
            nc.sync.dma_start(out=iota_s[:], in_=iota_d[:, :])

            f32r = dt.float32r
            relu = mybir.ActivationFunctionType.Relu
            iseq = mybir.AluOpType.is_equal
            mult = mybir.AluOpType.mult

            for rep in range(reps):
                call_ctr = 0
                for g in range(NG):
                    gsz = GROUP_SIZES[g]
                    gb = int(grp_base[g])
                    ps = {}  # (t, s_loc // 4) -> [128, 512] psum quad tile
                    g_tile = None
                    cur_call = -1
                    for w in range(NW):
                        c0 = int(chunk_base[g, w, 0])
                        c1 = int(chunk_base[g, w, n_types - 1]
                                 + caps[g, w, n_types - 1])
                        for ci in range(c0, c1):
                            k = call_of_chunk[ci]
                            if k != cur_call:
                                cur_call = k
                                wn, col0, wd = calls[k]
                                b0 = wn * WBASE
                                b1 = min(b0 + WBASE, n_nodes)
                                qn = call_ctr % NQ
                                call_ctr += 1
                                g_tile = gpool.tile(
                                    [128, KG, 256], dt.bfloat16,
                                    tag=f"g{qn}", name="g")
                                nc.gpsimd.dma_gather(
                                    g_tile[:, :wd, :], hpk[b0:b1, :],
                                    gidx_s[:, col0 * 8:(col0 + wd) * 8],
                                    128 * wd, 128 * wd, 256,
                                    single_packet=False, queue_num=qn)
                            jj = ci - calls[k][1]
                            gg, ww, t = rt["chunk_cell"][ci]
                            for sl in subs_of_chunk[ci]:
                                j = sub_id[(ci, sl)]
                                skey = (t, gb + sl)
                                ind = ind_p.tile([128, 128], dt.bfloat16,
                                                 tag="ind", name="ind")
                                nc.vector.tensor_scalar(
                                    out=ind[:], in0=iota_s[:],
                                    scalar1=scl_s[:, 2 * j:2 * j + 1],
                                    scalar2=scl_s[:, 2 * j + 1:2 * j + 2],
                                    op0=iseq, op1=mult)
                                qd = sl // 4
                                if (t, qd) not in ps:
                                    ps[(t, qd)] = psT_p.tile(
                                        [128, 512], dt.float32,
                                        tag=f"ps{t}_{qd}", name=f"ps{t}_{qd}")
                                co = (sl % 4) * 128
                                pt = ps[(t, qd)][:, co:co + 128]
                                st = first_q[(t, g, qd)] == j
                                sp = last_q[(t, g, qd)] == j
                                nc.tensor.matmul(
                                    out=pt, lhsT=g_tile[:, jj, 0:128],
                                    rhs=ind[:], start=st, stop=False)
                                nc.tensor.matmul(
                                    out=pt, lhsT=g_tile[:, jj, 128:256],
                                    rhs=ind[:], start=False, stop=sp)

                    # ---------------- finalize group: weight matmuls + out
                    for pl in range(gsz // 2):
                        s0 = gb + 2 * pl
                        q2 = s0 // 2
                        mts = []
                        for t in range(n_types):
                            mt = mt_p.tile([128, 256], f32r, tag=f"mt{t}",
                                           name=f"mt{t}")
                            qd, co = pl // 2, (pl % 2) * 256
                            nc.scalar.copy(out=mt[:],
                                           in_=ps[(t, qd)][:, co:co + 256])
                            mts.append(mt)
                        hot_t = hot_p.tile([128, 256], f32r, tag="hot",
                                           name="hot_t")
                        nc.sync.dma_start(
                            out=hot_t[:],
                            in_=hot_d[:, q2 * 256:(q2 + 1) * 256])
                        pso = pso_p.tile([128, 256], dt.float32, tag="pso",
                                         name="pso")
                        nc.tensor.matmul(out=pso[:], lhsT=w_s[0][:],
                                         rhs=mts[0][:], start=True,
                                         stop=False)
                        nc.tensor.matmul(out=pso[:], lhsT=w_s[1][:],
                                         rhs=mts[1][:], start=False,
                                         stop=False)
                        nc.tensor.matmul(out=pso[:], lhsT=w_s[2][:],
                                         rhs=hot_t[:], start=False,
                                         stop=True)
                        ot = o_p.tile([128, 256], dt.float32, tag="ot",
                                      name="ot")
                        nc.scalar.activation(out=ot[:], in_=pso[:],
                                             func=relu, bias=blc_s[:, 0:1])
                        nc.sync.dma_start(
                            out=outT_d[:, q2 * 256:(q2 + 1) * 256],
                            in_=ot[:])

    nc.compile()
    return nc


# ------------------------------------------------------------------ driver ---

def _prepare(h, src1, dst1, src2, dst2, W1, W2, Wl, bl,
             rows_per_core=ROWS_PER_CORE, n_cores=N_CORES):
    h = np.asarray(h, np.float32)
    bl = np.asarray(bl, np.float32)
    srcs = [np.asarray(src1), np.asarray(src2)]
    dsts = [np.asarray(dst1), np.asarray(dst2)]
    rt = _route(srcs, dsts)

    hi = h.astype(BF16)
    lo = (h - hi.astype(np.float32)).astype(BF16)
    hpk = np.concatenate([hi, lo], axis=1)  # [N, 256] bf16

    w1t = (0.5 * np.asarray(W1, np.float32).T).copy()
    w2t = (0.5 * np.asarray(W2, np.float32).T).copy()
    wlt = np.asarray(Wl, np.float32).T.copy()
    blc = bl.reshape(128, 1).copy()
    iota = np.broadcast_to(np.arange(128, dtype=np.float32), (128, 128))
    iota = np.ascontiguousarray(iota.astype(BF16))

    in_maps = []
    for c in range(n_cores):
        pc = rt["per_core"][c]
        rows = h[c * rows_per_core:(c + 1) * rows_per_core]
        pad = S * 128 - rows.shape[0]
        rows = np.pad(rows, ((0, pad), (0, 0)))
        hot = np.ascontiguousarray(rows.T)  # [128, S*128]
        in_maps.append(dict(
            hpk=hpk, gidx=pc["gidx"], scl=pc["scl"], hot=hot,
            w1t=w1t, w2t=w2t, wlt=wlt, blc=blc, iota=iota,
        ))
    return rt, in_maps


def _postprocess(results, rt, rows_per_core=ROWS_PER_CORE, n_cores=N_CORES):
    n_nodes = rows_per_core * n_cores
    out = np.empty((n_nodes, HIDDEN), np.float32)
    for c in range(n_cores):
        outT = results[c]["outT"]  # [128, S*128]
        out[c * rows_per_core:(c + 1) * rows_per_core] = \
            outT[:, :rows_per_core].T
    return out


def kernel(h, src1, dst1, src2, dst2, W1, W2, Wl, bl, **kw):
    from concourse import bass_utils
    rt, in_maps = _prepare(h, src1, dst1, src2, dst2, W1, W2, Wl, bl)
    nc = _build_program(rt, N_NODES, N_CORES)
    res = bass_utils.run_bass_kernel_spmd(
        nc, in_maps, core_ids=list(range(N_CORES)))
    return _postprocess(res.results, rt)


# revision 11
# speedup vs baseline: 2.7981x; 1.1811x over previous
"""GCN layer (2 edge types, mean aggregation + self-loop) on 8 Trainium2 cores.

Math (per reference):
    m_t = segment_mean(h[src_t] @ Wt.T, dst_t)   for t in {1,2}
    out = relu(h @ Wl.T + bl + 0.5*(m1 + m2))

Linear commutes with gather+mean: raw h rows are segment-mean'd first and
the 128x128 weights applied afterwards.

Design (v3) — measured bottlenecks drive everything:
  * dma_gather costs ~8ns/descriptor (row) regardless of row size, and
    4 SWDGE queues run near-parallel -> minimize descriptors, spread
    calls round-robin over 4 queues, gather single-bf16 rows (256B).
  * dst nodes partitioned contiguously across 8 cores (12500 = 98 slots
    of 128).  Slots processed in 9 PSUM-resident groups (8x12 + 2);
    each (type, quad-of-4-slots) owns one PSUM bank ([128, 512] f32)
    alive across all 4 src windows -> no SBUF accumulator traffic.
    Only the bank's globally-first matmul sets start (a start clears
    accumulate-bits bank-wide), only its last sets stop.
  * Segment-sum as a flipped indicator matmul
        psT[f, d] += sum_e g[e, f] * ind[e, d]
    (matmul(lhsT=g_chunk, rhs=ind)) giving the transposed mean directly.
    ind = is_equal(iota, drel) on DVE (unfused: a fused second ALU op
    measures 2x slower).  Chunks spanning two adjacent slots of a quad
    use one 256-wide indicator instead of two 128-wide ones.
  * The 1/deg mean scale is applied on the PSUM->SBUF move: one DVE
    tensor_tensor mult per (type, slot-pair) against a DMA-loaded
    partition-replicated inv table.
  * Edges routed to (core, type, group, window) cells, packed densely
    into chunks of 128 (93% fill); src indices int16 relative to one of
    4 windows of 25000 rows.
  * Final per slot-pair: 3 float32r matmuls (256-wide, full PE rate):
        out.T = relu(0.5*W1.T' m1T + 0.5*W2.T' m2T + Wl.T' hT + bl)

All 8 cores share one SPMD instruction stream; the chunk/subchunk
schedule is the max-shape over cores, per-core tables (gather indices,
drel scalar columns, inv tables) specialize it.  Padding slots gather
window row 0 and carry a sentinel drel -> indicator 0.
"""

import numpy as np
import ml_dtypes

BF16 = np.dtype(ml_dtypes.bfloat16)

# ---------------------------------------------------------------- config ---

N_NODES = 100000
HIDDEN = 128
N_CORES = 8
ROWS_PER_CORE = N_NODES // N_CORES  # 12500
S = 98                        # dst slots per core (12544 >= 12500)
GROUP_SIZES = [12] * 8 + [2]  # PSUM-resident slot groups
NW = 4                        # src windows
WBASE = 25000                 # window w covers rows [w*WBASE, (w+1)*WBASE)
KG = 4                        # chunks per dma_gather call
NQ = 4                        # SWDGE queues
SCRATCH = 32768               # dynamic DMA descriptor carveout
SENT = 999.0                  # drel sentinel -> indicator 0


def _cdiv(a, b):
    return -(-a // b)


# ------------------------------------------------------------ host routing ---

def _route(srcs, dsts):
    """Build the shared (static) chunk/subchunk schedule + per-core tables."""
    NG = len(GROUP_SIZES)
    grp_base = np.concatenate([[0], np.cumsum(GROUP_SIZES)[:-1]])
    grp_of = np.repeat(np.arange(NG), GROUP_SIZES)  # slot -> group

    n_types = len(srcs)
    invdeg = []
    for t in range(n_types):
        deg = np.bincount(dsts[t].astype(np.int64), minlength=N_NODES)
        invdeg.append((1.0 / np.maximum(deg, 1)).astype(np.float32))

    ed = []
    for t in range(n_types):
        src = srcs[t].astype(np.int64)
        dst = dsts[t].astype(np.int64)
        c = dst // ROWS_PER_CORE
        dl = dst - c * ROWS_PER_CORE
        s = dl >> 7
        d128 = (dl & 127).astype(np.float32)
        g = grp_of[s]
        w = src // WBASE
        idx16 = (src - w * WBASE).astype(np.int16)
        order = np.lexsort((src, s, w, g, c))
        ed.append(dict(c=c[order], s=s[order], d128=d128[order],
                       g=g[order], w=w[order], idx16=idx16[order]))

    gmax = max(GROUP_SIZES)
    cnt = np.zeros((n_types, N_CORES, NG, NW), np.int64)
    cnt_s = np.zeros((n_types, N_CORES, NG, NW, gmax), np.int64)
    for t in range(n_types):
        e = ed[t]
        s_loc = e["s"] - grp_base[e["g"]]
        np.add.at(cnt[t], (e["c"], e["g"], e["w"]), 1)
        np.add.at(cnt_s[t], (e["c"], e["g"], e["w"], s_loc), 1)

    caps = np.zeros((NG, NW, n_types), np.int64)
    for g in range(NG):
        for w in range(NW):
            for t in range(n_types):
                caps[g, w, t] = _cdiv(int(cnt[t][:, g, w].max()), 128)
    for g in range(NG):
        for t in range(n_types):
            if caps[g, :, t].sum() == 0:
                caps[g, 0, t] = 1

    chunk_base = np.zeros((NG, NW, n_types), np.int64)
    pos = 0
    for g in range(NG):
        for w in range(NW):
            for t in range(n_types):
                chunk_base[g, w, t] = pos
                pos += int(caps[g, w, t])
    n_chunks = pos

    calls = []  # (window, col0, width)
    for g in range(NG):
        for w in range(NW):
            c0 = int(chunk_base[g, w, 0])
            c1 = int(chunk_base[g, w, n_types - 1] + caps[g, w, n_types - 1])
            c = c0
            while c < c1:
                wd = min(KG, c1 - c)
                calls.append((w, c, wd))
                c += wd

    # per-chunk union (over cores) of spanned local slots
    slots_of_chunk = [set() for _ in range(n_chunks)]
    for g in range(NG):
        gsz = GROUP_SIZES[g]
        for w in range(NW):
            for t in range(n_types):
                Q = int(caps[g, w, t])
                if Q == 0:
                    continue
                base = int(chunk_base[g, w, t])
                for c in range(N_CORES):
                    cum = 0
                    for sl in range(gsz):
                        n = int(cnt_s[t][c, g, w, sl])
                        if n == 0:
                            continue
                        q0, q1 = cum // 128, (cum + n - 1) // 128
                        for q in range(q0, q1 + 1):
                            slots_of_chunk[base + q].add(sl)
                        cum += n

    # coverage injection for (t, s) with no edges anywhere
    covered = np.zeros((n_types, S), bool)
    for g in range(NG):
        for w in range(NW):
            for t in range(n_types):
                base = int(chunk_base[g, w, t])
                for q in range(int(caps[g, w, t])):
                    for sl in slots_of_chunk[base + q]:
                        covered[t, grp_base[g] + sl] = True
    for t in range(n_types):
        for s in range(S):
            if not covered[t, s]:
                g = int(grp_of[s])
                for w in range(NW):
                    if caps[g, w, t] > 0:
                        base = int(chunk_base[g, w, t])
                        slots_of_chunk[base].add(s - int(grp_base[g]))
                        break

    # merge adjacent slots (within a quad) into 256-wide subchunks
    # subs_of_chunk[ci] = [(sl_lo, n_slots)]; cover[(ci, sl)] = (j, sl_lo)
    subs_of_chunk = [[] for _ in range(n_chunks)]
    for ci in range(n_chunks):
        SL = sorted(slots_of_chunk[ci])
        i = 0
        while i < len(SL):
            sl = SL[i]
            if (i + 1 < len(SL) and SL[i + 1] == sl + 1 and sl % 4 < 3):
                subs_of_chunk[ci].append((sl, 2))
                i += 2
            else:
                subs_of_chunk[ci].append((sl, 1))
                i += 1

    sub_id = {}     # (ci, sl_lo) -> j
    cover = {}      # (ci, sl) -> (j, sl_lo)
    chunk_cell = [None] * n_chunks
    first_q, last_q = {}, {}   # (t, g, quad) -> j
    j = 0
    for g in range(NG):
        for w in range(NW):
            for t in range(n_types):
                base = int(chunk_base[g, w, t])
                for q in range(int(caps[g, w, t])):
                    ci = base + q
                    chunk_cell[ci] = (g, w, t)
                    for (sl, ns) in subs_of_chunk[ci]:
                        sub_id[(ci, sl)] = j
                        for k in range(ns):
                            cover[(ci, sl + k)] = (j, sl)
                        qkey = (t, g, sl // 4)
                        if qkey not in first_q:
                            first_q[qkey] = j
                        last_q[qkey] = j
                        j += 1
    n_sub = j

    # ------------------------------------------------------ per-core tables
    per_core = []
    for c in range(N_CORES):
        flat_idx = np.zeros(n_chunks * 128, np.int16)
        scl = np.full((128, n_sub), SENT, np.float32)
        for t in range(n_types):
            e = ed[t]
            mask = e["c"] == c
            idx = np.nonzero(mask)[0]
            if len(idx) == 0:
                continue
            gs, ws = e["g"][idx], e["w"][idx]
            sl = e["s"][idx] - grp_base[gs]
            cellkey = gs * NW + ws
            bounds = np.nonzero(np.diff(cellkey))[0] + 1
            starts = np.concatenate([[0], bounds])
            ends = np.concatenate([bounds, [len(idx)]])
            for lo, hi in zip(starts, ends):
                g, w = int(gs[lo]), int(ws[lo])
                base = int(chunk_base[g, w, t])
                p = np.arange(hi - lo)
                eidx = idx[lo:hi]
                flat_idx[base * 128 + p] = e["idx16"][eidx]
                slr = sl[lo:hi]
                rb = np.nonzero(np.diff(slr))[0] + 1
                rst = np.concatenate([[0], rb])
                ren = np.concatenate([rb, [hi - lo]])
                for a, b in zip(rst, ren):
                    s_loc = int(slr[a])
                    for q in range(a // 128, (b - 1) // 128 + 1):
                        pa, pb = max(a, q * 128), min(b, (q + 1) * 128)
                        jj, sl_lo = cover[(base + q, s_loc)]
                        rows = np.arange(pa, pb) % 128
                        sel = eidx[pa:pb]
                        scl[rows, jj] = (e["d128"][sel]
                                         + 128.0 * (s_loc - sl_lo))

        gidx = np.zeros((128, n_chunks * 8), np.int16)
        for (w, col0, wd) in calls:
            seg = flat_idx[col0 * 128:(col0 + wd) * 128]
            gidx[:, col0 * 8:(col0 + wd) * 8] = \
                np.tile(seg.reshape(-1, 16).T, (8, 1))

        invb = []
        for t in range(n_types):
            row = np.zeros(S * 128, np.float32)
            row[:ROWS_PER_CORE] = invdeg[t][c * ROWS_PER_CORE:
                                            (c + 1) * ROWS_PER_CORE]
            invb.append(np.ascontiguousarray(
                np.broadcast_to(row, (128, S * 128))))
        per_core.append(dict(gidx=np.ascontiguousarray(gidx), scl=scl,
                             invb=invb))

    return dict(caps=caps, chunk_base=chunk_base, n_chunks=n_chunks,
                n_sub=n_sub, calls=calls, subs_of_chunk=subs_of_chunk,
                sub_id=sub_id, chunk_cell=chunk_cell,
                first_q=first_q, last_q=last_q,
                grp_base=grp_base, per_core=per_core)


# ------------------------------------------------------------ bass program ---

def _build_program(rt, n_nodes, n_cores, reps=1):
    import os
    import concourse.bacc as bacc
    from concourse import mybir, tile, library_config

    mode = os.environ.get("KMODE", "full")  # full | gather | noind | nomm
    NG = len(GROUP_SIZES)
    n_types = 2
    caps, chunk_base = rt["caps"], rt["chunk_base"]
    n_chunks, n_sub = rt["n_chunks"], rt["n_sub"]
    calls, subs_of_chunk = rt["calls"], rt["subs_of_chunk"]
    sub_id = rt["sub_id"]
    first_q, last_q = rt["first_q"], rt["last_q"]
    grp_base = rt["grp_base"]

    nc = bacc.Bacc("TRN2", target_bir_lowering=False, debug=False,
                   num_devices=n_cores, dynamic_dma_scratch_size=SCRATCH,
                   num_swdge_queues=NQ)
    dt = mybir.dt

    hpk = nc.dram_tensor("hpk", [n_nodes, 128], dt.bfloat16,
                         kind="ExternalInput").ap()
    gidx_d = nc.dram_tensor("gidx", [128, n_chunks * 8], dt.int16,
                            kind="ExternalInput").ap()
    scl_d = nc.dram_tensor("scl", [128, n_sub], dt.float32,
                           kind="ExternalInput").ap()
    invb_d = [nc.dram_tensor(f"invb{t}", [128, S * 128], dt.float32,
                             kind="ExternalInput").ap()
              for t in range(n_types)]
    hot_d = nc.dram_tensor("hot", [128, S * 128], dt.float32r,
                           kind="ExternalInput").ap()
    w_d = [nc.dram_tensor(w, [128, 128], dt.float32r,
                          kind="ExternalInput").ap()
           for w in ("w1t", "w2t", "wlt")]
    blc_d = nc.dram_tensor("blc", [128, 1], dt.float32,
                           kind="ExternalInput").ap()
    iota_d = nc.dram_tensor("iota", [128, 256], dt.bfloat16,
                            kind="ExternalInput").ap()
    outT_d = nc.dram_tensor("outT", [128, S * 128], dt.float32,
                            kind="ExternalOutput").ap()

    call_of_chunk = {}
    for k, (w, col0, wd) in enumerate(calls):
        for ci in range(col0, col0 + wd):
            call_of_chunk[ci] = k

    with tile.TileContext(nc) as tc:
        with (
            tc.tile_pool(name="const", bufs=1) as const_p,
            tc.tile_pool(name="gpool", bufs=6) as gpool,
            tc.tile_pool(name="ind", bufs=8) as ind_p,
            tc.tile_pool(name="mt", bufs=2) as mt_p,
            tc.tile_pool(name="invb", bufs=2) as invb_p,
            tc.tile_pool(name="hot", bufs=2) as hot_p,
            tc.tile_pool(name="ostage", bufs=2) as o_p,
            tc.tile_pool(name="psT", bufs=1, space="PSUM") as psT_p,
            tc.tile_pool(name="pso", bufs=2, space="PSUM") as pso_p,
        ):
            nc.gpsimd.load_library(library_config.mlp)
            gidx_s = const_p.tile([128, n_chunks * 8], dt.int16, name="gidx_s")
            nc.sync.dma_start(out=gidx_s[:], in_=gidx_d[:, :])
            scl_s = const_p.tile([128, n_sub], dt.float32, name="scl_s")
            nc.sync.dma_start(out=scl_s[:], in_=scl_d[:, :])
            w_s = []
            for i, wd_ in enumerate(w_d):
                wt = const_p.tile([128, 128], dt.float32r, tag=f"w{i}",
                                  name=f"ws{i}")
                nc.sync.dma_start(out=wt[:], in_=wd_[:, :])
                w_s.append(wt)
            blc_s = const_p.tile([128, 1], dt.float32, name="blc_s")
            nc.sync.dma_start(out=blc_s[:], in_=blc_d[:, :])
            iota_s = const_p.tile([128, 256], dt.bfloat16, name="iota_s")
            nc.sync.dma_start(out=iota_s[:], in_=iota_d[:, :])

            f32r = dt.float32r
            relu = mybir.ActivationFunctionType.Relu
            iseq = mybir.AluOpType.is_equal
            mult = mybir.AluOpType.mult

            for rep in range(reps):
                call_ctr = 0
                for g in range(NG):
                    gsz = GROUP_SIZES[g]
                    gb = int(grp_base[g])
                    # inv tables for this group (overlaps with gathers)
                    invb_s = []
                    for t in range(n_types):
                        iv = invb_p.tile([128, gsz * 128], dt.float32,
                                         tag=f"invb{t}", name=f"invb{t}")
                        nc.sync.dma_start(
                            out=iv[:],
                            in_=invb_d[t][:, gb * 128:(gb + gsz) * 128])
                        invb_s.append(iv)
                    ps = {}  # (t, quad) -> [128, 512] psum bank tile
                    g_tile = None
                    cur_call = -1
                    for w in range(NW):
                        c0 = int(chunk_base[g, w, 0])
                        c1 = int(chunk_base[g, w, n_types - 1]
                                 + caps[g, w, n_types - 1])
                        for ci in range(c0, c1):
                            k = call_of_chunk[ci]
                            if k != cur_call:
                                cur_call = k
                                wn, col0, wd = calls[k]
                                b0 = wn * WBASE
                                b1 = min(b0 + WBASE, n_nodes)
                                qn = call_ctr % NQ
                                call_ctr += 1
                                g_tile = gpool.tile(
                                    [128, KG, 128], dt.bfloat16,
                                    tag=f"g{qn}", name="g")
                                nc.gpsimd.dma_gather(
                                    g_tile[:, :wd, :], hpk[b0:b1, :],
                                    gidx_s[:, col0 * 8:(col0 + wd) * 8],
                                    128 * wd, 128 * wd, 128,
                                    single_packet=False, queue_num=qn)
                            jj = ci - calls[k][1]
                            gg, ww, t = rt["chunk_cell"][ci]
                            if mode == "gather":
                                continue
                            for (sl, ns) in subs_of_chunk[ci]:
                                j = sub_id[(ci, sl)]
                                wide = ns * 128
                                ind = ind_p.tile([128, 256], dt.bfloat16,
                                                 tag="ind", name="ind")
                                if mode != "noind":
                                    nc.vector.tensor_scalar(
                                        out=ind[:, :wide],
                                        in0=iota_s[:, :wide],
                                        scalar1=scl_s[:, j:j + 1],
                                        scalar2=None, op0=iseq)
                                if mode == "nomm":
                                    continue
                                qd = sl // 4
                                if (t, qd) not in ps:
                                    ps[(t, qd)] = psT_p.tile(
                                        [128, 512], dt.float32,
                                        tag=f"ps{t}_{qd}", name=f"ps{t}_{qd}")
                                co = (sl % 4) * 128
                                st = first_q[(t, g, qd)] == j
                                sp = last_q[(t, g, qd)] == j
                                nc.tensor.matmul(
                                    out=ps[(t, qd)][:, co:co + wide],
                                    lhsT=g_tile[:, jj, :],
                                    rhs=ind[:, :wide], start=st, stop=sp)

                    # ---------------- finalize group: weight matmuls + out
                    if mode in ("gather", "nomm"):
                        continue
                    for pl in range(gsz // 2):
                        q2 = (gb + 2 * pl) // 2
                        mts = []
                        for t in range(n_types):
                            mt = mt_p.tile([128, 256], f32r, tag=f"mt{t}",
                                           name=f"mt{t}")
                            qd, co = pl // 2, (pl % 2) * 256
                            nc.vector.tensor_tensor(
                                out=mt[:], in0=ps[(t, qd)][:, co:co + 256],
                                in1=invb_s[t][:, pl * 256:(pl + 1) * 256],
                                op=mult)
                            mts.append(mt)
                        hot_t = hot_p.tile([128, 256], f32r, tag="hot",
                                           name="hot_t")
                        nc.sync.dma_start(
                            out=hot_t[:],
                            in_=hot_d[:, q2 * 256:(q2 + 1) * 256])
                        pso = pso_p.tile([128, 256], dt.float32, tag="pso",
                                         name="pso")
                        nc.tensor.matmul(out=pso[:], lhsT=w_s[0][:],
                                         rhs=mts[0][:], start=True,
                                         stop=False)
                        nc.tensor.matmul(out=pso[:], lhsT=w_s[1][:],
                                         rhs=mts[1][:], start=False,
                                         stop=False)
                        nc.tensor.matmul(out=pso[:], lhsT=w_s[2][:],
                                         rhs=hot_t[:], start=False,
                                         stop=True)
                        ot = o_p.tile([128, 256], dt.float32, tag="ot",
                                      name="ot")
                        nc.scalar.activation(out=ot[:], in_=pso[:],
                                             func=relu, bias=blc_s[:, 0:1])
                        nc.sync.dma_start(
                            out=outT_d[:, q2 * 256:(q2 + 1) * 256],
                            in_=ot[:])

    nc.compile()
    return nc


# ------------------------------------------------------------------ driver ---

def _prepare(h, src1, dst1, src2, dst2, W1, W2, Wl, bl,
             rows_per_core=ROWS_PER_CORE, n_cores=N_CORES):
    h = np.asarray(h, np.float32)
    bl = np.asarray(bl, np.float32)
    srcs = [np.asarray(src1), np.asarray(src2)]
    dsts = [np.asarray(dst1), np.asarray(dst2)]
    rt = _route(srcs, dsts)

    hpk = np.ascontiguousarray(h.astype(BF16))  # [N, 128] bf16

    w1t = (0.5 * np.asarray(W1, np.float32).T).copy()
    w2t = (0.5 * np.asarray(W2, np.float32).T).copy()
    wlt = np.asarray(Wl, np.float32).T.copy()
    blc = bl.reshape(128, 1).copy()
    iota = np.broadcast_to(np.arange(256, dtype=np.float32), (128, 256))
    iota = np.ascontiguousarray(iota.astype(BF16))

    in_maps = []
    for c in range(n_cores):
        pc = rt["per_core"][c]
        rows = h[c * rows_per_core:(c + 1) * rows_per_core]
        pad = S * 128 - rows.shape[0]
        rows = np.pad(rows, ((0, pad), (0, 0)))
        hot = np.ascontiguousarray(rows.T)  # [128, S*128]
        in_maps.append(dict(
            hpk=hpk, gidx=pc["gidx"], scl=pc["scl"],
            invb0=pc["invb"][0], invb1=pc["invb"][1], hot=hot,
            w1t=w1t, w2t=w2t, wlt=wlt, blc=blc, iota=iota,
        ))
    return rt, in_maps


def _postprocess(results, rt, rows_per_core=ROWS_PER_CORE, n_cores=N_CORES):
    n_nodes = rows_per_core * n_cores
    out = np.empty((n_nodes, HIDDEN), np.float32)
    for c in range(n_cores):
        outT = results[c]["outT"]  # [128, S*128]
        out[c * rows_per_core:(c + 1) * rows_per_core] = \
            outT[:, :rows_per_core].T
    return out


def kernel(h, src1, dst1, src2, dst2, W1, W2, Wl, bl, **kw):
    from concourse import bass_utils
    rt, in_maps = _prepare(h, src1, dst1, src2, dst2, W1, W2, Wl, bl)
    nc = _build_program(rt, N_NODES, N_CORES)
    res = bass_utils.run_bass_kernel_spmd(
        nc, in_maps, core_ids=list(range(N_CORES)))
    return _postprocess(res.results, rt)
